# revision 52
# baseline (speedup 1.0000x reference)
"""BasicYATBlock kernel for Trainium2 (Bass/Tile), data-parallel over batch on 8 cores.

Computes, per sample (stride=2 block, 128ch 56x56 -> 256ch 28x28):
    identity = conv1x1_s2(x, w_short)
    dot      = conv3x3_s2_p1(x, w_yat)
    patch_sq = conv3x3_s2_p1(x*x, ones)          (per-patch squared norm)
    yat      = dot^2 / (patch_sq + |w|^2 - 2 dot + EPS) * scale
    out      = conv3x3_s1_p1(yat, w_lin) + identity

All three convs are TensorE matmuls. The two big convs run in fp8 with
DoubleRow perf mode (2 K-tiles per instruction at 0.5 PE cycles/row).
DoubleRow ifmaps must be [p][pair 2][N] access patterns with
non-overlapping pair windows (overlapping windows abort at runtime), so
layouts are built to make every tap window one contiguous 392-byte run:
  - x is uploaded as six 29x28 parity sub-planes (row parity hh x three
    column alignments kw=0/1/2, zero padding baked in on the host), row-
    split into a rows 0..15 half and a rows 14..28 half so the two output
    chunks can be fed independently. A stride-2 3x3 tap is then rows
    a0..a0+13 x all 28 cols = one flat [off, off+392) run. conv1 pairs
    2 taps per matmul (pair 5 = tap9 + zero weights on a dummy window).
  - patch_sq: same tap pairs over xsq = x^2, shared across both co-tiles.
    mean(|w|^2)+eps rides into the same PSUM group as a K=1 DoubleRow
    matmul against an all-ones rhs (per-co |w|^2 variation is <0.04% of
    dist^2, far below fp8 noise), so no separate bias op is needed.
  - conv2 DoubleRow-pairs the two ci tiles (K=256 per instruction) on a
    32x28 row-padded yat plane; kw!=1 taps read one column out of line,
    which wraps to the adjacent row's edge column - those spurious terms
    are accumulated by 6 tiny N=14 correction matmuls into scratch at the
    end of the same PSUM bank and subtracted from the output's edge
    columns after the PSUM->SBUF copy.
The identity shortcut stays bf16 (it dominates the output magnitude;
fp8 there would eat most of the 2e-2 error budget). Scale bookkeeping:
x8=e4m3(x), wyat8=e4m3(16 w), xsq=e4m3(x^2), d = (pt * -1/8) + pp
= dist^2+eps; num = pt^2 = 256 dot^2; yat8 = e4m3(256 yat);
wlin8 = e5m2(w_lin*alpha_scale/256). The 1/256 keeps wlin roughly in
e5m2 normal range (reaches 6e-5) and the 256 keeps yat8 in e4m3 range.

Engine budget (hardware rules: GPSIMD cannot touch PSUM; other engines
may read at most one PSUM operand per instruction):
  PE   ~27.5us  all matmuls (398 of them, mostly DoubleRow)
  ACT  ~24us    x squares (shares), pp->SBUF copies, num=pt^2, out copies
  DVE  ~24us    d (scalar_tensor_tensor), 1/d, edge-col fixups, sq shares
  Pool ~19us    x*x squares (bulk), yat=num*r multiplies, memsets, SWDGE
  SP   ~19us    x/out DMA queue (issue cost ~= transfer bytes * 0.39ns)
The scheduler dispatches ready-first per engine, so dep-free weight DMAs
are emitted on queues whose idle windows match (wyat/wk1 on SWDGE at the
head; wlin/wshort on the ACT queue, WAR-gated behind the first squares
so they cannot jump ahead of PE-feeding work). Squares are split across
ACT/DVE/Pool per sample to keep the patch_sq feed ahead of the PE.
Cost-model time: ~35.8us/core (2.4x the fp32r baseline's 86.1us).
"""

import numpy as np
import ml_dtypes

import bass_rust
import concourse.bass as bass
import concourse.bacc as bacc
import concourse.mybir as mybir
from concourse import tile
from concourse.bass_utils import run_bass_kernel_spmd

F32 = mybir.dt.float32
BF16 = mybir.dt.bfloat16
F8E4 = mybir.dt.float8e4
F8E5 = mybir.dt.float8e5

E4 = ml_dtypes.float8_e4m3
E5 = ml_dtypes.float8_e5m2
BF = ml_dtypes.bfloat16

N_CORES = 8
NPER = 4          # samples per core
CI = 128          # input channels
CO = 256          # output channels (2 tiles of 128)
H = 56            # input spatial
OH = 28           # output spatial
CH = 14           # output rows per chunk
NCH = 2           # chunks per plane
NPIX = CH * OH    # 392 free elements per PSUM tile
PBANK = NPIX + 2 * CH  # psum slot: 392 out + 2x14 wrap-correction scratch
HALF0 = 6 * 16 * OH    # 2688: rows 0..15 of the six sub-planes
HALF1 = 6 * 15 * OH    # 2520: rows 14..28
XSZ = HALF0 + HALF1    # 5208 per sample
YROWS = 32        # yat plane rows (2 top pad + 28 + 2 bottom pad)
YSZ = YROWS * OH  # 896
EPS = 0.007

# conv1/patch_sq tap pairs (tap index = kh*3+kw); pair 5 = (tap 8, dummy)
TAP_PAIRS = [(0, 1), (2, 3), (4, 5), (6, 7), (8, None)]
# conv2 correction taps: kw=0 group then kw=2 group
COR_TAPS = [0, 3, 6, 2, 5, 8]
WARMUP_MMS = 10
DR = mybir.MatmulPerfMode.DoubleRow


def _xoff(kh, kw, c):
    """Flat offset of stride-2 tap (kh,kw), chunk c in the half-split
    6-sub-plane x layout."""
    pl = (kh % 2) * 3 + kw
    a0 = c * CH + (1 if kh == 2 else 0)
    if c == 0:
        return pl * (16 * OH) + a0 * OH
    return HALF0 + pl * (15 * OH) + (a0 - CH) * OH


def _rhs3(flat_ap, off, part_dim, pair_stride, n, nstride=1):
    """3-dim DoubleRow rhs AP [p][pair 2][n]."""
    d = flat_ap.copy()
    d.ap = bass_rust.VecI64Pair([list(part_dim), [pair_stride, 2],
                                 [nstride, n]])
    d.offset = flat_ap.offset + off
    return d


def build_nc(nc=None, loop_n=1):
    if nc is None:
        nc = bass.Bass()

    x8_d = nc.dram_tensor("x8", [NPER, CI, XSZ], F8E4, kind="ExternalInput")
    xq_d = nc.dram_tensor("xq", [CI, NPER, OH * OH], BF16, kind="ExternalInput")
    wyat_d = nc.dram_tensor("wyat8", [CI, 5, 2, CO], F8E4, kind="ExternalInput")
    wlin_d = nc.dram_tensor("wlin8", [CI, 2, 9, CO], F8E5, kind="ExternalInput")
    wshort_d = nc.dram_tensor("wshort", [CI, CO], BF16, kind="ExternalInput")
    wk1_d = nc.dram_tensor("wk1", [1, 2, CO], F8E4, kind="ExternalInput")
    out_d = nc.dram_tensor("out", [NPER, CI, 2, OH * OH], F32,
                           kind="ExternalOutput")

    with tile.TileContext(nc) as tc:
        with (
            tc.tile_pool(name="const", bufs=1) as const,
            tc.tile_pool(name="xsqp", bufs=2) as xsqp,
            tc.tile_pool(name="scr", bufs=4) as scr,
            tc.tile_pool(name="outp", bufs=2) as outp,
            tc.tile_pool(name="psum", bufs=8, space="PSUM") as psum,
        ):
            wyat_sb = const.tile([CI, 5, 2, CO], F8E4, tag="wyat")
            wlin_sb = const.tile([CI, 2, 9, CO], F8E5, tag="wlin")
            wshort_sb = const.tile([CI, CO], BF16, tag="wshort")
            wk1_sb = const.tile([1, 2, CO], F8E4, tag="wk1")
            ones_sb = const.tile([CI, 2, CI], F8E4, tag="ones")
            onez_sb = const.tile([CI, 2, CI], F8E4, tag="onez")
            ones1_sb = const.tile([1, 2 * NPIX], F8E4, tag="ones1")
            x8_sb = [const.tile([CI, XSZ], F8E4, tag=f"x{s}", name=f"x8_{s}")
                     for s in range(NPER)]
            xq_sb = const.tile([CI, NPER, OH * OH], BF16, tag="xq")
            yat_sb = [const.tile([CI, 2, YSZ], F8E4, tag=f"yat{s}",
                                 name=f"yat_{s}") for s in range(NPER)]

            def emit_iter(_it=0):
                XS = {}   # per-sample xsq tile
                PT = {}   # per-sample dot psums [c][t]
                PP = {}   # per-sample patch_sq psums [c]

                # head: constants + first loads
                nc.vector.memset(ones1_sb[:, :1], 1.0)
                nc.gpsimd.memset(ones_sb[:], 1.0)
                nc.vector.memset(ones1_sb[:, 1:], 1.0)
                nc.gpsimd.memset(onez_sb[:, 0], 1.0)
                nc.gpsimd.memset(onez_sb[:, 1], 0.0)
                if _it == 0:
                    # prime the ACT Square table (~1.3us) during the idle
                    # head; square(1)=1 keeps the ones tile intact
                    nc.scalar.square(ones1_sb[:, :1], ones1_sb[:, :1])
                nc.gpsimd.dma_start(out=wyat_sb[:], in_=wyat_d[:])
                nc.gpsimd.dma_start(out=wk1_sb[:], in_=wk1_d[:])
                nc.gpsimd.dma_start(out=wlin_sb[:], in_=wlin_d[:])
                nc.gpsimd.dma_start(out=wshort_sb[:], in_=wshort_d[:])
                nc.sync.dma_start(out=x8_sb[0][:, :HALF0],
                                  in_=x8_d[0, :, :HALF0])
                nc.sync.dma_start(out=x8_sb[0][:, HALF0:],
                                  in_=x8_d[0, :, HALF0:])
                nc.sync.dma_start(out=x8_sb[1][:], in_=x8_d[1])
                if _it == 0:
                    # keep the PE p-state ramp warm through the DMA head
                    pw = psum.tile([CI, PBANK], F32, tag="ps", name="pwarm")
                    ones_flat = ones_sb[:].rearrange("p a b -> p (a b)")
                    for _w in range(WARMUP_MMS):
                        nc.tensor.matmul(pw[:, :2 * CI], ones_sb[:, 0],
                                         ones_flat, start=True, stop=True)

                def prep(s):
                    """xsq = x^2 in e4m3. Sample 0 splits the two row-halves
                    across ACT and Pool so both chunks' patch_sq can start
                    as early as possible; padding squares to 0."""
                    xsq = xsqp.tile([CI, XSZ], F8E4, tag="xsq",
                                    name=f"xsq{s}")
                    XS[s] = xsq
                    sq = mybir.ActivationFunctionType.Square
                    if s == 0:
                        mid = HALF0 + HALF1 // 2
                        nc.scalar.activation(xsq[:, :HALF0],
                                             x8_sb[s][:, :HALF0], sq)
                        nc.vector.tensor_mul(out=xsq[:, HALF0:mid],
                                             in0=x8_sb[s][:, HALF0:mid],
                                             in1=x8_sb[s][:, HALF0:mid])
                        nc.gpsimd.tensor_mul(out=xsq[:, mid:],
                                             in0=x8_sb[s][:, mid:],
                                             in1=x8_sb[s][:, mid:])
                    elif s == 1:
                        mid = HALF0 + HALF1 // 2
                        nc.gpsimd.tensor_mul(out=xsq[:, :HALF0],
                                             in0=x8_sb[s][:, :HALF0],
                                             in1=x8_sb[s][:, :HALF0])
                        nc.scalar.activation(xsq[:, HALF0:mid],
                                             x8_sb[s][:, HALF0:mid], sq)
                        nc.vector.tensor_mul(out=xsq[:, mid:],
                                             in0=x8_sb[s][:, mid:],
                                             in1=x8_sb[s][:, mid:])
                    else:
                        nc.gpsimd.tensor_mul(out=xsq[:, :HALF0],
                                             in0=x8_sb[s][:, :HALF0],
                                             in1=x8_sb[s][:, :HALF0])
                        nc.gpsimd.tensor_mul(out=xsq[:, HALF0:],
                                             in0=x8_sb[s][:, HALF0:],
                                             in1=x8_sb[s][:, HALF0:])
                    # yat plane pad rows (top 2, bottom 2)
                    for t in range(2):
                        nc.gpsimd.memset(yat_sb[s][:, t, :2 * OH], 0.0)
                        nc.gpsimd.memset(yat_sb[s][:, t, 30 * OH:YSZ], 0.0)

                def dp(s):
                    """conv1 dot + patch_sq matmuls."""
                    xflat = x8_sb[s][:]
                    qflat = XS[s][:]
                    part = xflat.ap[0]
                    qpart = qflat.ap[0]
                    pt = [[None, None], [None, None]]
                    pp = [None, None]
                    PT[s], PP[s] = pt, pp
                    for c in range(NCH):
                        for t in range(2):
                            ptile = psum.tile([CI, PBANK], F32, tag="ps",
                                              name=f"pt{s}_{c}_{t}")
                            pt[c][t] = ptile
                            for pi, (ta, tb) in enumerate(TAP_PAIRS):
                                oa = _xoff(ta // 3, ta % 3, c)
                                ob = (oa + NPIX if tb is None
                                      else _xoff(tb // 3, tb % 3, c))
                                rhs = _rhs3(xflat, oa, part, ob - oa, NPIX)
                                lhsT = wyat_sb[:, pi, :, t * CI:(t + 1) * CI]
                                nc.tensor.matmul(
                                    ptile[:, :NPIX], lhsT, rhs, perf_mode=DR,
                                    start=(pi == 0), stop=(pi == 4))
                        # patch_sq: shared across co-tiles
                        pptile = psum.tile([CI, PBANK], F32, tag="ps",
                                           name=f"pp{s}_{c}")
                        pp[c] = pptile
                        for pi, (ta, tb) in enumerate(TAP_PAIRS):
                            oa = _xoff(ta // 3, ta % 3, c)
                            ob = (oa + NPIX if tb is None
                                  else _xoff(tb // 3, tb % 3, c))
                            rhs = _rhs3(qflat, oa, qpart, ob - oa, NPIX)
                            lhsT = (onez_sb if tb is None else ones_sb)[:]
                            nc.tensor.matmul(
                                pptile[:, :NPIX], lhsT, rhs, perf_mode=DR,
                                start=(pi == 0), stop=False)
                        # + mean(|w|^2)+eps via a K=1 DoubleRow vs all-ones
                        # rhs (per-co wsq variation is <0.04% of dist^2)
                        nc.tensor.matmul(
                            pptile[:, :NPIX], wk1_sb[:, :, :CI],
                            ones1_sb[:].rearrange("p (k n) -> p k n", k=2),
                            perf_mode=DR, start=False, stop=True)

                def chain(s):
                    """YAT elementwise: psqe -> d -> 1/d -> num -> yat8."""
                    pt, pp = PT[s], PP[s]
                    for c in range(NCH):
                        pq = scr.tile([CI, NPIX], F32, tag="q")
                        nc.scalar.copy(pq[:], pp[c][:, :NPIX])
                        for t in range(2):
                            d4 = scr.tile([CI, NPIX], F32, tag="d")
                            r4 = scr.tile([CI, NPIX], F32, tag="r")
                            num = scr.tile([CI, NPIX], F32, tag="n")
                            nc.vector.scalar_tensor_tensor(
                                out=d4[:], in0=pt[c][t][:, :NPIX],
                                scalar=-0.125, in1=pq[:],
                                op0=mybir.AluOpType.mult,
                                op1=mybir.AluOpType.add)
                            nc.vector.reciprocal_approx_fast(out=r4[:],
                                                             in_=d4[:])
                            nc.scalar.square(num[:], pt[c][t][:, :NPIX])
                            ybase = 2 * OH + c * NPIX
                            if c == 0:
                                nc.gpsimd.tensor_mul(
                                    out=yat_sb[s][:, t, ybase:ybase + NPIX],
                                    in0=num[:], in1=r4[:])
                            else:
                                # rows 14-15 first: conv2's chunk-0 blocks
                                # read up to yat row 15, so this unblocks
                                # them without waiting the whole chunk
                                nc.gpsimd.tensor_mul(
                                    out=yat_sb[s][:, t, ybase:ybase + 2 * OH],
                                    in0=num[:, :2 * OH], in1=r4[:, :2 * OH])
                                nc.gpsimd.tensor_mul(
                                    out=yat_sb[s][:, t, ybase + 2 * OH:
                                                  ybase + NPIX],
                                    in0=num[:, 2 * OH:], in1=r4[:, 2 * OH:])

                def phase_b(s):
                    """conv2 (3x3 s1 p1, fp8 DR over ci pairs, wrap-corrected)
                    + bf16 1x1 s2 shortcut -> out."""
                    yflat = yat_sb[s][:].rearrange("p a b -> p (a b)")
                    ypart = yflat.ap[0]
                    last = s == NPER - 1
                    out_t = outp.tile([CI, 4 * NPIX], F32, tag="out")
                    for t in range(2):
                        for c in range(NCH):
                            po = psum.tile([CI, PBANK], F32, tag="ps",
                                           name=f"po{s}_{t}_{c}")
                            po3 = po[:, :NPIX].rearrange(
                                "p (r q) -> p r q", q=OH)
                            nc.tensor.matmul(
                                po[:, :NPIX],
                                wshort_sb[:, t * CI:(t + 1) * CI],
                                xq_sb[:, s, c * NPIX:(c + 1) * NPIX],
                                start=True, stop=False)
                            for ti in range(9):
                                kh, kw = ti // 3, ti % 3
                                off = (c * CH + kh + 1) * OH + kw - 1
                                rhs = _rhs3(yflat, off, ypart, YSZ, NPIX)
                                lhsT = wlin_sb[:, :, ti, t * CI:(t + 1) * CI]
                                nc.tensor.matmul(
                                    po[:, :NPIX], lhsT, rhs,
                                    perf_mode=DR, start=False, stop=False)
                            # accumulate the column-wrap spurious terms into
                            # the same bank's scratch region (contiguous psum
                            # writes only), then subtract into the edge cols
                            for ci_, ti in enumerate(COR_TAPS):
                                kh, kw = ti // 3, ti % 3
                                if kw == 0:
                                    off = (c * CH + kh) * OH + OH - 1
                                    oview = po[:, NPIX:NPIX + CH]
                                else:
                                    off = (c * CH + kh + 2) * OH
                                    oview = po[:, NPIX + CH:NPIX + 2 * CH]
                                rhs = _rhs3(yflat, off, ypart, YSZ, CH,
                                            nstride=OH)
                                lhsT = wlin_sb[:, :, ti,
                                               t * CI:(t + 1) * CI]
                                nc.tensor.matmul(
                                    oview, lhsT, rhs, perf_mode=DR,
                                    start=False, stop=(ci_ == 5))
                            base = t * 2 * NPIX + c * NPIX
                            dst = out_t[:, base:base + NPIX]
                            if t == 0:
                                nc.scalar.copy(dst, po[:, :NPIX])
                            else:
                                nc.vector.tensor_copy(out=dst,
                                                      in_=po[:, :NPIX])
                            # subtract the wrap scratch into the edge cols
                            # of the SBUF copy (PSUM-pair reads are illegal)
                            ecols = out_t[:].copy()
                            ecols.ap = bass_rust.VecI64Pair(
                                [list(out_t[:].ap[0]), [OH, CH], [OH - 1, 2]])
                            ecols.offset = out_t[:].offset + base
                            spv = po[:].copy()
                            spv.ap = bass_rust.VecI64Pair(
                                [list(po[:].ap[0]), [1, CH], [CH, 2]])
                            spv.offset = po[:].offset + NPIX
                            nc.vector.tensor_sub(out=ecols, in0=ecols,
                                                 in1=spv)
                            if last:
                                # spread the tail stores across HWDGE queues
                                q = [nc.sync, nc.scalar,
                                     nc.sync, nc.scalar][t * 2 + c]
                                q.dma_start(
                                    out=out_d[s, :, t,
                                              c * NPIX:(c + 1) * NPIX],
                                    in_=dst)
                    if not last:
                        nc.sync.dma_start(
                            out=out_d[s].rearrange("c t x -> c (t x)"),
                            in_=out_t[:])

                # software pipeline: PE order dp0,dp1,B0,dp2,B1,dp3,B2,B3;
                # ACT order sq0,sq1,sq2,cp0,sq3,cp1,cp2,cp3 (squares are on
                # the PE-feed path and must not queue behind output copies)
                prep(0)
                dp(0)
                chain(0)
                nc.sync.dma_start(out=x8_sb[2][:], in_=x8_d[2])
                prep(1)
                dp(1)
                chain(1)
                nc.sync.dma_start(out=x8_sb[3][:], in_=x8_d[3])
                nc.sync.dma_start(out=xq_sb[:], in_=xq_d[:])
                prep(2)
                phase_b(0)
                dp(2)
                chain(2)
                prep(3)
                phase_b(1)
                dp(3)
                chain(3)
                phase_b(2)
                phase_b(3)

            for _it in range(loop_n):
                emit_iter(_it)

    return nc


_NC_CACHE = {}


def _get_nc(loop_n=1):
    key = loop_n
    if key not in _NC_CACHE:
        nc = bacc.Bacc(None, target_bir_lowering=False)
        build_nc(nc=nc, loop_n=loop_n)
        nc.compile()
        _NC_CACHE[key] = nc
    return _NC_CACHE[key]


def out_to_full(arr):
    """[NPER, CI, 2, OH*OH] device layout -> [NPER, CO, OH, OH]."""
    return np.ascontiguousarray(arr.transpose(0, 2, 1, 3)).reshape(
        arr.shape[0], CO, OH, OH)


def prep_inputs(x, w_yat, alpha, w_lin, w_short):
    """Host-side dtype/layout prep for the full batch."""
    x = np.asarray(x, np.float32)
    w_yat = np.asarray(w_yat, np.float32)
    w_lin = np.asarray(w_lin, np.float32)
    w_short = np.asarray(w_short, np.float32)
    n = x.shape[0]

    # six 29x28 parity sub-planes with padding baked in:
    # plane (hh, kw): rows = padded rows hh,hh+2,..,hh+56; cols per kw:
    # kw=0: padded cols 0,2,..,54; kw=1: 1,3,..,55; kw=2: 2,4,..,56;
    # then row-split into rows 0..15 and rows 14..28 halves
    xpad = np.zeros((n, CI, 58, 58), np.float32)
    xpad[:, :, 1:H + 1, 1:H + 1] = x
    planes = np.empty((n, CI, 6, 29, OH), np.float32)
    for hh in range(2):
        rows = xpad[:, :, hh:hh + 58:2, :]
        planes[:, :, hh * 3 + 0] = rows[:, :, :, 0:56:2]
        planes[:, :, hh * 3 + 1] = rows[:, :, :, 1:57:2]
        planes[:, :, hh * 3 + 2] = rows[:, :, :, 2:58:2]
    p8 = planes.astype(E4)
    x8 = np.concatenate(
        [p8[:, :, :, 0:16, :].reshape(n, CI, HALF0),
         p8[:, :, :, 14:29, :].reshape(n, CI, HALF1)], axis=2)
    # xq: [CI, NPER-per-core..., pix] so all samples ride one DMA per core
    xq = np.ascontiguousarray(
        x[:, :, ::2, ::2].reshape(n, CI, OH * OH).transpose(1, 0, 2)
    ).astype(BF)

    # conv1 weights: x16, tap-paired [ci, pair, 2, co]
    wt = np.ascontiguousarray(
        (w_yat * np.float32(16.0)).transpose(1, 2, 3, 0)).reshape(CI, 9, CO)
    wyat8 = np.zeros((CI, 5, 2, CO), E4)
    wyat8[:, :4] = wt[:, :8].reshape(CI, 4, 2, CO).astype(E4)
    wyat8[:, 4, 0] = wt[:, 8].astype(E4)

    scale = float((np.sqrt(np.float32(CO)) / np.log1p(np.float32(CO)))
                  ** np.float32(np.asarray(alpha).ravel()[0]))
    wlin_t = np.ascontiguousarray(
        (w_lin * np.float32(scale / 256.0)).transpose(1, 2, 3, 0)
    ).reshape(2, CI, 9, CO).transpose(1, 0, 2, 3)
    wlin8 = np.ascontiguousarray(wlin_t).astype(E5)  # [ci, ci_tile, tap, co]

    wshort = np.ascontiguousarray(
        w_short[:, :, 0, 0].transpose(1, 0)).astype(BF)

    wsq = (w_yat.astype(np.float32) ** 2).sum(axis=(1, 2, 3))
    wk1 = np.zeros((1, 2, CO), E4)
    wk1[0, 0] = np.float32(wsq.mean() + EPS)

    return {"x8": x8, "xq": xq, "wyat8": wyat8, "wlin8": wlin8,
            "wshort": wshort, "wk1": wk1}


def kernel(x, w_yat, alpha, w_lin, w_short, _trace=False):
    import os
    # this axon deployment has no NTFF hook (antenv.axon_hooks absent);
    # make sure an inherited BASS_TRACE can't route us into that path
    if not _trace:
        os.environ["BASS_NEVER_TRACE"] = "1"
    full = prep_inputs(x, w_yat, alpha, w_lin, w_short)
    nc = _get_nc()
    in_maps = []
    for i in range(N_CORES):
        m = {k: v for k, v in full.items() if k not in ("x8", "xq")}
        m["x8"] = full["x8"][i * NPER:(i + 1) * NPER]
        m["xq"] = np.ascontiguousarray(
            full["xq"][:, i * NPER:(i + 1) * NPER])
        in_maps.append(m)
    res = run_bass_kernel_spmd(nc, in_maps, core_ids=list(range(N_CORES)),
                               trace=_trace)
    out = np.concatenate([out_to_full(res.results[i]["out"])
                          for i in range(N_CORES)], axis=0)
    if _trace:
        kernel.last_results = res
    return out
